# revision 1
# baseline (speedup 1.0000x reference)
"""GNN edge-softmax message-passing kernel for 8 Trainium2 NeuronCores.

Problem (see reference):
    z1 = rel[src] * pattern                       # [E, D]
    e  = leaky_relu(z1 @ w1 + rel[dst] @ w2)      # [E]
    alpha = segment_softmax(e, by dst)            # [E]
    agg   = segment_sum(alpha[:, None] * z1, dst) # [N, D]
    out   = where(deg > 0, agg, rel)

Sharding strategy (dst-ownership, no collectives):
    Every dst node is assigned to exactly one (core, block, partition)
    slot.  Nodes are sorted by in-degree and packed into 128-node blocks
    so all nodes in a block have (nearly) the same degree K.  A block's
    edges live in a [128, K, D] slab where partition p holds the edges of
    the block's p-th node.  Segment max / sum / softmax then become
    per-partition row reductions - there is no scatter and no cross-core
    reduction at all.  Blocks are dealt round-robin to the 8 cores so all
    cores share one compiled program (same K schedule).

    While sharding the edge arrays the host also lays the rel[src] rows
    out in the same edge-slot order (the device DGE gather paths bottom
    out in per-256B descriptor generation or int16 index limits for a
    100k-row table), so every device-side DMA is a contiguous line-rate
    stream and the NeuronCores run all of the model compute: attention
    logits, leaky-relu, segment max/softmax, weighted aggregation and the
    zero-degree fallback.
"""

import math
import numpy as np

import concourse.bacc as bacc
import concourse.tile as tile
from concourse import mybir
from concourse.bass_utils import run_bass_kernel_spmd

P = 128
NCORES = 8
D = 64

f32 = mybir.dt.float32


# ---------------------------------------------------------------------------
# Host-side preprocessing
# ---------------------------------------------------------------------------

def _host_prep(rel, pattern, src, dst, ncores):
    """Pack nodes/edges into the per-core block layout.

    Returns a dict with per-core input arrays, the shared K schedule, and
    the slot->node mapping needed to unpermute the output.
    """
    N = rel.shape[0]
    E = src.shape[0]

    deg = np.bincount(dst, minlength=N).astype(np.int64)

    # Degree-descending node order; blocks of P nodes then get ~uniform K.
    node_order = np.argsort(-deg, kind="stable")

    group = P * ncores                       # nodes per row of blocks
    B = int(math.ceil(N / group))            # blocks per core
    total_slots = B * group

    slot_node = np.full(total_slots, -1, dtype=np.int64)
    slot_node[:N] = node_order

    deg_slot = np.zeros(total_slots, dtype=np.int64)
    deg_slot[:N] = deg[node_order]

    # K_j = max degree within block-group j.
    Ks = deg_slot.reshape(B, group).max(axis=1).astype(np.int64)

    offs = np.zeros(B + 1, dtype=np.int64)        # column offsets per block
    offs[1:] = np.cumsum(Ks)
    sumK = int(Ks.sum())

    # --- edge -> (core, block, partition, k) ------------------------------
    slot_of_node = np.empty(N, dtype=np.int64)
    slot_of_node[node_order] = np.arange(N)

    e_slot = slot_of_node[dst]                    # [E]
    order = np.argsort(e_slot, kind="stable")
    es_sorted = e_slot[order]
    counts = np.bincount(e_slot, minlength=total_slots)
    starts = np.concatenate([[0], np.cumsum(counts)[:-1]])
    k_sorted = np.arange(E, dtype=np.int64) - starts[es_sorted]

    g_sorted = es_sorted // P
    p_sorted = es_sorted % P
    c_sorted = g_sorted % ncores
    j_sorted = g_sorted // ncores

    addr_sorted = (offs[j_sorted] * P) + p_sorted * Ks[j_sorted] + k_sorted

    src_sorted = src[order]
    patt_rows_sorted = order                      # row index into pattern

    tot_i = P * sumK                              # edge slots per core
    cores = []
    for c in range(ncores):
        msk = c_sorted == c
        addr_c = addr_sorted[msk]
        patt_c = np.zeros((tot_i, D), dtype=np.float32)
        patt_c[addr_c] = pattern[patt_rows_sorted[msk]]
        hsrc_c = np.zeros((tot_i, D), dtype=np.float32)
        hsrc_c[addr_c] = rel[src_sorted[msk]]

        gsel = (np.arange(total_slots) // P) % ncores == c
        nodes_c = slot_node[gsel]                 # [B*P], -1 for pads
        deg_c = deg_slot[gsel].astype(np.float32)
        relperm = np.zeros((B * P, D), dtype=np.float32)
        valid = nodes_c >= 0
        relperm[valid] = rel[nodes_c[valid]]

        cores.append(
            dict(
                patt=patt_c.reshape(-1),
                hsrc=hsrc_c.reshape(-1),
                relperm=relperm,
                deg=deg_c,
                nodes=nodes_c,
            )
        )

    return dict(cores=cores, Ks=Ks, offs=offs, B=B, sumK=sumK)


# ---------------------------------------------------------------------------
# Device program
# ---------------------------------------------------------------------------

def _build_program(Ks, offs, d=D):
    """Build the SPMD Bass program (identical on every core)."""
    B = len(Ks)
    sumK = int(offs[-1])
    kmax = int(max(int(Ks.max()), 1))
    nper = B * P

    nc = bacc.Bacc("TRN2", target_bir_lowering=False)

    relperm_t = nc.dram_tensor("relperm", [nper, d], f32, kind="ExternalInput")
    patt_t = nc.dram_tensor("patt", [P * sumK * d], f32, kind="ExternalInput")
    hsrc_t = nc.dram_tensor("hsrc", [P * sumK * d], f32, kind="ExternalInput")
    deg_t = nc.dram_tensor("deg", [nper], f32, kind="ExternalInput")
    wattn_t = nc.dram_tensor("wattn", [2 * d], f32, kind="ExternalInput")
    out_t = nc.dram_tensor("out", [nper, d], f32, kind="ExternalOutput")

    with tile.TileContext(nc) as tc:
        with (
            tc.tile_pool(name="const", bufs=1) as cpool,
            tc.tile_pool(name="big", bufs=2) as bpool,
            tc.tile_pool(name="small", bufs=2) as spool,
        ):
            # ---- one-time constants ----
            w_row = cpool.tile([1, 2 * d], f32, tag="w_row")
            nc.sync.dma_start(w_row[:], wattn_t[:].rearrange("(p f) -> p f", p=1))
            w_all = cpool.tile([P, 2 * d], f32, tag="w_all")
            nc.gpsimd.partition_broadcast(w_all[:], w_row[:])

            iota_i = cpool.tile([P, kmax], mybir.dt.int32, tag="iota_i")
            nc.gpsimd.iota(iota_i[:], pattern=[[1, kmax]], channel_multiplier=0)
            iota_f = cpool.tile([P, kmax], f32, tag="iota_f")
            nc.vector.tensor_copy(iota_f[:], iota_i[:])

            for j in range(B):
                K = int(Ks[j])
                relp = spool.tile([P, d], f32, tag="relp")
                nc.sync.dma_start(relp[:], relperm_t[j * P:(j + 1) * P, :])
                outb = spool.tile([P, d], f32, tag="outb")

                if K == 0:
                    nc.vector.tensor_copy(outb[:], relp[:])
                    nc.sync.dma_start(out_t[j * P:(j + 1) * P, :], outb[:])
                    continue

                ioff = int(offs[j]) * P
                patt = bpool.tile([P, K, d], f32, tag="patt")
                nc.sync.dma_start(
                    patt[:],
                    patt_t[ioff * d:(ioff + P * K) * d].rearrange(
                        "(p k f) -> p k f", p=P, k=K
                    ),
                )
                hsrc = bpool.tile([P, K, d], f32, tag="hsrc")
                nc.sync.dma_start(
                    hsrc[:],
                    hsrc_t[ioff * d:(ioff + P * K) * d].rearrange(
                        "(p k f) -> p k f", p=P, k=K
                    ),
                )
                degc = spool.tile([P, 1], f32, tag="degc")
                nc.sync.dma_start(
                    degc[:], deg_t[j * P:(j + 1) * P].rearrange("(p f) -> p f", f=1)
                )

                # prod = hsrc * patt
                prod = bpool.tile([P, K, d], f32, tag="prod")
                nc.vector.tensor_tensor(
                    out=prod[:], in0=hsrc[:], in1=patt[:], op=mybir.AluOpType.mult
                )

                # zw = prod * w1  (w1 broadcast over k) -> reuse hsrc slab
                w1b = w_all[:, :d].unsqueeze(1).to_broadcast([P, K, d])
                nc.vector.tensor_tensor(
                    out=hsrc[:], in0=prod[:], in1=w1b, op=mybir.AluOpType.mult
                )

                # logits = reduce_d zw
                logits = spool.tile([P, K], f32, tag="logits")
                nc.vector.tensor_reduce(
                    out=logits[:], in_=hsrc[:], axis=mybir.AxisListType.X,
                    op=mybir.AluOpType.add,
                )

                # q = reduce_d relp * w2   [P, 1]
                qtmp = spool.tile([P, d], f32, tag="qtmp")
                nc.vector.tensor_tensor(
                    out=qtmp[:], in0=relp[:], in1=w_all[:, d:2 * d],
                    op=mybir.AluOpType.mult,
                )
                qcol = spool.tile([P, 1], f32, tag="qcol")
                nc.vector.tensor_reduce(
                    out=qcol[:], in_=qtmp[:], axis=mybir.AxisListType.X,
                    op=mybir.AluOpType.add,
                )

                # logits += q ; lrelu
                nc.vector.tensor_scalar(
                    out=logits[:], in0=logits[:], scalar1=qcol[:, :1], scalar2=None,
                    op0=mybir.AluOpType.add,
                )
                l01 = spool.tile([P, K], f32, tag="l01")
                nc.vector.tensor_scalar(
                    out=l01[:], in0=logits[:], scalar1=0.01, scalar2=None,
                    op0=mybir.AluOpType.mult,
                )
                nc.vector.tensor_tensor(
                    out=logits[:], in0=logits[:], in1=l01[:], op=mybir.AluOpType.max
                )

                # negm = -max_k logits ; ex = exp(logits - m) * padmask
                negm = spool.tile([P, 1], f32, tag="negm")
                nc.vector.tensor_reduce(
                    out=negm[:], in_=logits[:], axis=mybir.AxisListType.X,
                    op=mybir.AluOpType.max, negate=True,
                )
                ex = spool.tile([P, K], f32, tag="ex")
                nc.scalar.activation(
                    out=ex[:], in_=logits[:],
                    func=mybir.ActivationFunctionType.Exp,
                    bias=negm[:, :1], scale=1.0,
                )
                mask = spool.tile([P, K], f32, tag="mask")
                nc.vector.tensor_scalar(
                    out=mask[:], in0=iota_f[:, :K], scalar1=degc[:, :1], scalar2=None,
                    op0=mybir.AluOpType.is_lt,
                )
                nc.vector.tensor_tensor(
                    out=ex[:], in0=ex[:], in1=mask[:], op=mybir.AluOpType.mult
                )

                # s = sum_k ex
                scol = spool.tile([P, 1], f32, tag="scol")
                nc.vector.tensor_reduce(
                    out=scol[:], in_=ex[:], axis=mybir.AxisListType.X,
                    op=mybir.AluOpType.add,
                )

                # ext = prod * ex (ex broadcast over d) -> reuse patt slab
                exb = ex[:].unsqueeze(2).to_broadcast([P, K, d])
                nc.vector.tensor_tensor(
                    out=patt[:], in0=prod[:], in1=exb, op=mybir.AluOpType.mult
                )

                # agg = sum_k ext   (reduce innermost after transpose view)
                agg = spool.tile([P, d], f32, tag="agg")
                nc.vector.tensor_reduce(
                    out=agg[:], in_=patt[:].transpose([0, 2, 1]),
                    axis=mybir.AxisListType.X, op=mybir.AluOpType.add,
                )

                # normalize + deg==0 fallback
                sclamp = spool.tile([P, 1], f32, tag="sclamp")
                nc.vector.tensor_scalar(
                    out=sclamp[:], in0=scol[:], scalar1=1e-30, scalar2=None,
                    op0=mybir.AluOpType.max,
                )
                rcp = spool.tile([P, 1], f32, tag="rcp")
                nc.vector.reciprocal(rcp[:], sclamp[:])

                posm = spool.tile([P, 1], f32, tag="posm")
                nc.vector.tensor_scalar(
                    out=posm[:], in0=degc[:], scalar1=0.0, scalar2=None,
                    op0=mybir.AluOpType.is_gt,
                )
                invm = spool.tile([P, 1], f32, tag="invm")
                nc.vector.tensor_scalar(
                    out=invm[:], in0=posm[:], scalar1=-1.0, scalar2=1.0,
                    op0=mybir.AluOpType.mult, op1=mybir.AluOpType.add,
                )

                # out = agg * rcp * posm + relp * invm
                nc.vector.tensor_scalar(
                    out=agg[:], in0=agg[:], scalar1=rcp[:, :1], scalar2=posm[:, :1],
                    op0=mybir.AluOpType.mult, op1=mybir.AluOpType.mult,
                )
                nc.vector.tensor_scalar(
                    out=outb[:], in0=relp[:], scalar1=invm[:, :1], scalar2=None,
                    op0=mybir.AluOpType.mult,
                )
                nc.vector.tensor_tensor(
                    out=outb[:], in0=outb[:], in1=agg[:], op=mybir.AluOpType.add
                )
                nc.sync.dma_start(out_t[j * P:(j + 1) * P, :], outb[:])

    nc.compile()
    return nc


# ---------------------------------------------------------------------------
# Entry point
# ---------------------------------------------------------------------------

_last_results = None  # BassKernelResults of the most recent run (for profiling)


def kernel(rel, pattern, w_attn, src, dst, **_unused):
    rel = np.ascontiguousarray(np.asarray(rel, dtype=np.float32))
    pattern = np.ascontiguousarray(np.asarray(pattern, dtype=np.float32))
    w_attn = np.ascontiguousarray(np.asarray(w_attn, dtype=np.float32))
    src = np.asarray(src).astype(np.int64)
    dst = np.asarray(dst).astype(np.int64)

    prep = _host_prep(rel, pattern, src, dst, NCORES)
    Ks, offs = prep["Ks"], prep["offs"]

    nc = _build_program(Ks, offs)

    in_maps = []
    for c in range(NCORES):
        pc = prep["cores"][c]
        in_maps.append(
            dict(
                relperm=pc["relperm"],
                patt=pc["patt"],
                hsrc=pc["hsrc"],
                deg=pc["deg"],
                wattn=w_attn,
            )
        )

    res = run_bass_kernel_spmd(nc, in_maps, core_ids=list(range(NCORES)))
    global _last_results
    _last_results = res

    out = np.empty((rel.shape[0], D), dtype=np.float32)
    for c in range(NCORES):
        nodes_c = prep["cores"][c]["nodes"]
        valid = nodes_c >= 0
        out[nodes_c[valid]] = res.results[c]["out"][valid]
    return out



# revision 2
# speedup vs baseline: 1.8017x; 1.8017x over previous
"""GNN edge-softmax message-passing kernel for 8 Trainium2 NeuronCores.

Problem (see reference):
    z1 = rel[src] * pattern                       # [E, D]
    e  = leaky_relu(z1 @ w1 + rel[dst] @ w2)      # [E]
    alpha = segment_softmax(e, by dst)            # [E]
    agg   = segment_sum(alpha[:, None] * z1, dst) # [N, D]
    out   = where(deg > 0, agg, rel)

Sharding strategy (dst-ownership, no collectives):
    Every dst node is assigned to one (core, block, partition) slot.
    Nodes are degree-sorted and packed into 128-node blocks so all nodes
    in a block share the same padded edge count K; blocks are dealt
    round-robin to the 8 cores so all cores run one compiled program.
    Blocks of equal-ish K are fused into supergroups of G blocks
    (G*K <= GKMAX) so device instructions are few and large.

Device data layout ("layout B", k innermost):
    slab[p, g, d, k] fp16 for both gathered-node and pattern slabs, so
    every bulk DVE op keeps a packed (stride-1) innermost dim and runs
    in the 2x half-precision mode. All reductions are computed as
    tensor_tensor halving trees (2x mode) instead of tensor_reduce
    (1x mode only, slower still on strided views).

Algebra: w1 is folded into the node table before the host gather
    (hsrcw = (rel*w1)[src]), so  zw = hsrcw*patt  yields logits by a
    d-tree, and the weighted message sum reuses zw:
        agg' = sum_k alpha * zw = w1 * agg,
    un-scaled at the end by 1/w1 on the [P,G,64] result. Relative fp16
    error is invariant to the w1 scaling.

Pad slots are poisoned on host (hsrcw[d0] = -3000, patt[d0] = 1) so pad
    logits are ~-3000, leaky-relu -> -30, exp -> fp16 exact 0: no masks,
    no segment max (logits are O(5), exp cannot overflow), no degree
    correction. Zero-in-degree rows come out all-zero and the DGL
    fallback is a single add of a host-prepared `relout` (= rel where
    deg==0 else 0).

The scalar (ACT) engine runs leaky-relu(+q bias) and exp.
"""

import math
import numpy as np

import concourse.bacc as bacc
import concourse.tile as tile
from concourse import mybir
from concourse.bass_utils import run_bass_kernel_spmd

P = 128
NCORES = 8
D = 64
GKMAX = 192

f32 = mybir.dt.float32
f16 = mybir.dt.float16


# ---------------------------------------------------------------------------
# Host-side preprocessing
# ---------------------------------------------------------------------------

def _host_prep(rel, pattern, w_attn, src, dst, ncores):
    N = rel.shape[0]
    E = src.shape[0]

    deg = np.bincount(dst, minlength=N).astype(np.int64)
    node_order = np.argsort(-deg, kind="stable")

    group = P * ncores
    B = int(math.ceil(N / group))
    total_slots = B * group

    slot_node = np.full(total_slots, -1, dtype=np.int64)
    slot_node[:N] = node_order
    deg_slot = np.zeros(total_slots, dtype=np.int64)
    deg_slot[:N] = deg[node_order]
    Ks = deg_slot.reshape(B, group).max(axis=1).astype(np.int64)

    # supergroups of consecutive blocks, padded to the first (max) K
    sgs = []  # (jstart, G, K)
    j = 0
    while j < B:
        K = max(int(Ks[j]), 1)
        G = 1
        while j + G < B and (G + 1) * K <= GKMAX:
            G += 1
        sgs.append((j, G, K))
        j += G

    # per-edge coordinates (edges sorted by dst slot, k within node)
    slot_of_node = np.empty(N, dtype=np.int64)
    slot_of_node[node_order] = np.arange(N)
    e_slot = slot_of_node[dst]
    order = np.argsort(e_slot, kind="stable")
    es = e_slot[order]
    counts = np.bincount(e_slot, minlength=total_slots)
    starts = np.concatenate([[0], np.cumsum(counts)[:-1]])
    k_all = np.arange(E, dtype=np.int64) - starts[es]
    gg = es // P
    p_all = es % P
    c_all = (gg % ncores).astype(np.int64)
    j_all = gg // ncores
    src_all = src[order]
    prow_all = order

    relw = (rel * w_attn[None, :D]).astype(np.float32)  # w1 folded into table

    cores = []
    for c in range(ncores):
        mc = c_all == c
        hs_parts, pt_parts, rq_parts, ro_parts = [], [], [], []
        nodes_parts = []
        for (j0, G, K) in sgs:
            msk = mc & (j_all >= j0) & (j_all < j0 + G)
            pe = p_all[msk]
            ge = j_all[msk] - j0
            ke = k_all[msk]

            hv = np.zeros((P, G, D, K), dtype=np.float16)
            pv = np.zeros((P, G, D, K), dtype=np.float16)
            hv[pe, ge, :, ke] = relw[src_all[msk]]
            pv[pe, ge, :, ke] = pattern[prow_all[msk]]

            slots = ((j0 + np.arange(G)[None, :]) * ncores + c) * P \
                + np.arange(P)[:, None]                      # [P, G]
            nd = slot_node[slots]
            dg = deg_slot[slots]
            pmask = np.arange(K)[None, None, :] >= dg[:, :, None]
            pi, gi, ki = np.nonzero(pmask)
            hv[pi, gi, 0, ki] = -3000.0
            pv[pi, gi, 0, ki] = 1.0

            qv = np.zeros((P, G, D), dtype=np.float16)
            ov = np.zeros((P, G, D), dtype=np.float16)
            valid = nd >= 0
            qv[valid] = rel[nd[valid]]
            zd = valid & (dg == 0)
            ov[zd] = rel[nd[zd]]

            hs_parts.append(hv.reshape(P, -1))
            pt_parts.append(pv.reshape(P, -1))
            rq_parts.append(qv.reshape(P, -1))
            ro_parts.append(ov.reshape(P, -1))
            nodes_parts.append(nd)

        cores.append(
            dict(
                hsrcw=np.ascontiguousarray(np.concatenate(hs_parts, axis=1)),
                patt=np.ascontiguousarray(np.concatenate(pt_parts, axis=1)),
                relq=np.ascontiguousarray(np.concatenate(rq_parts, axis=1)),
                relout=np.ascontiguousarray(np.concatenate(ro_parts, axis=1)),
                nodes=nodes_parts,
            )
        )

    return dict(cores=cores, sgs=sgs)


# ---------------------------------------------------------------------------
# Device program
# ---------------------------------------------------------------------------

def _build_program(sgs, d=D):
    total_cols = sum(G * d * K for (_, G, K) in sgs)
    totq = sum(G * d for (_, G, _) in sgs)

    nc = bacc.Bacc("TRN2", target_bir_lowering=False)

    hsrcw_t = nc.dram_tensor("hsrcw", [P, total_cols], f16, kind="ExternalInput")
    patt_t = nc.dram_tensor("patt", [P, total_cols], f16, kind="ExternalInput")
    relq_t = nc.dram_tensor("relq", [P, totq], f16, kind="ExternalInput")
    relout_t = nc.dram_tensor("relout", [P, totq], f16, kind="ExternalInput")
    wattn_t = nc.dram_tensor("wattn", [2 * d], f32, kind="ExternalInput")
    out_t = nc.dram_tensor("out", [P, totq], f16, kind="ExternalOutput")

    mult = mybir.AluOpType.mult
    add = mybir.AluOpType.add
    mx = mybir.AluOpType.max
    X = mybir.AxisListType.X
    Lrelu = mybir.ActivationFunctionType.Lrelu
    Exp = mybir.ActivationFunctionType.Exp

    with tile.TileContext(nc) as tc:
        with (
            tc.tile_pool(name="const", bufs=1) as cpool,
            tc.tile_pool(name="big", bufs=2) as bpool,
            tc.tile_pool(name="small", bufs=2) as spool,
        ):
            w_row = cpool.tile([1, 2 * d], f32, tag="w_row")
            nc.sync.dma_start(w_row[:], wattn_t[:].rearrange("(p f) -> p f", p=1))
            w_all = cpool.tile([P, 2 * d], f32, tag="w_all")
            nc.gpsimd.partition_broadcast(w_all[:], w_row[:])
            w2_16 = cpool.tile([P, 1, d], f16, tag="w2_16")
            nc.vector.tensor_copy(w2_16[:], w_all[:, d:].unsqueeze(1))
            w1inv = cpool.tile([P, 1, d], f32, tag="w1inv")
            nc.vector.reciprocal(w1inv[:], w_all[:, :d].unsqueeze(1))

            coff = 0
            qoff = 0
            with nc.allow_low_precision(reason="fp16 streams within tolerance"):
                for (j0, G, K) in sgs:
                    cols = G * d * K
                    qcols = G * d

                    hs = bpool.tile([P, G, d, K], f16, tag="hs")
                    hsf = hs[:].rearrange("p g e k -> p (g e k)")
                    for a, b in ((0, cols // 2), (cols // 2, cols)):
                        nc.sync.dma_start(hsf[:, a:b], hsrcw_t[:, coff + a:coff + b])
                    pt = bpool.tile([P, G, d, K], f16, tag="pt")
                    ptf = pt[:].rearrange("p g e k -> p (g e k)")
                    for a, b in ((0, cols // 2), (cols // 2, cols)):
                        nc.sync.dma_start(ptf[:, a:b], patt_t[:, coff + a:coff + b])
                    rq = spool.tile([P, G, d], f16, tag="rq")
                    nc.sync.dma_start(
                        rq[:].rearrange("p g e -> p (g e)"),
                        relq_t[:, qoff:qoff + qcols],
                    )
                    ro = spool.tile([P, G, d], f16, tag="ro")
                    nc.sync.dma_start(
                        ro[:].rearrange("p g e -> p (g e)"),
                        relout_t[:, qoff:qoff + qcols],
                    )

                    # zw = hsrcw * patt, in place over hs
                    nc.vector.tensor_tensor(out=hs[:], in0=hs[:], in1=pt[:], op=mult)

                    # logits = sum_d zw : halving tree over the d axis.
                    # level 1 -> scratch (zw must survive); further in place.
                    lt = bpool.tile([P, G, d // 2, K], f16, tag="lt")
                    nc.vector.tensor_tensor(
                        out=lt[:], in0=hs[:, :, :d // 2, :], in1=hs[:, :, d // 2:, :],
                        op=add,
                    )
                    w = d // 2
                    while w > 1:
                        h = w // 2
                        nc.vector.tensor_tensor(
                            out=lt[:, :, :h, :], in0=lt[:, :, :h, :],
                            in1=lt[:, :, h:2 * h, :], op=add,
                        )
                        if w % 2:
                            nc.vector.tensor_tensor(
                                out=lt[:, :, :1, :], in0=lt[:, :, :1, :],
                                in1=lt[:, :, w - 1:w, :], op=add,
                            )
                        w = h

                    # q = (relq * w2) summed over d
                    qt = spool.tile([P, G, d], f16, tag="qt")
                    nc.vector.tensor_tensor(
                        out=qt[:], in0=rq[:], in1=w2_16[:].to_broadcast([P, G, d]),
                        op=mult,
                    )
                    qq = spool.tile([P, G], f32, tag="qq")
                    nc.vector.tensor_reduce(out=qq[:], in_=qt[:], axis=X, op=add)

                    # ex = exp(leaky_relu(logits + q)); pads underflow to 0
                    el = spool.tile([P, G, K], f16, tag="el")
                    for g in range(G):
                        nc.scalar.activation(
                            out=el[:, g, :], in_=lt[:, g, 0, :], func=Lrelu,
                            bias=qq[:, g:g + 1], alpha=0.01,
                        )
                    ex = spool.tile([P, G, K], f16, tag="ex")
                    nc.scalar.activation(
                        out=ex[:].rearrange("p g k -> p (g k)"),
                        in_=el[:].rearrange("p g k -> p (g k)"), func=Exp,
                    )

                    # alpha = ex / sum_k ex   (in place over ex)
                    sc = spool.tile([P, G], f32, tag="sc")
                    nc.vector.tensor_reduce(out=sc[:], in_=ex[:], axis=X, op=add)
                    scl = spool.tile([P, G], f32, tag="scl")
                    nc.vector.tensor_scalar(
                        out=scl[:], in0=sc[:], scalar1=1e-30, scalar2=None, op0=mx
                    )
                    rc = spool.tile([P, G], f32, tag="rc")
                    nc.vector.reciprocal(rc[:], scl[:])
                    nc.vector.tensor_tensor(
                        out=ex[:], in0=ex[:],
                        in1=rc[:].unsqueeze(2).to_broadcast([P, G, K]), op=mult,
                    )

                    # ext = zw * alpha, in place over pt; then k halving tree
                    nc.vector.tensor_tensor(
                        out=pt[:], in0=hs[:],
                        in1=ex[:].unsqueeze(2).to_broadcast([P, G, d, K]), op=mult,
                    )
                    w = K
                    while w > 1:
                        h = w // 2
                        nc.vector.tensor_tensor(
                            out=pt[:, :, :, :h], in0=pt[:, :, :, :h],
                            in1=pt[:, :, :, h:2 * h], op=add,
                        )
                        if w % 2:
                            nc.vector.tensor_tensor(
                                out=pt[:, :, :, :1], in0=pt[:, :, :, :1],
                                in1=pt[:, :, :, w - 1:w], op=add,
                            )
                        w = h

                    # agg = agg' / w1 ; out = agg + relout
                    ag = spool.tile([P, G, d], f32, tag="ag")
                    nc.vector.tensor_tensor(
                        out=ag[:], in0=pt[:, :, :, 0],
                        in1=w1inv[:].to_broadcast([P, G, d]), op=mult,
                    )
                    ob = spool.tile([P, G, d], f16, tag="ob")
                    nc.vector.tensor_tensor(out=ob[:], in0=ag[:], in1=ro[:], op=add)
                    nc.sync.dma_start(
                        out_t[:, qoff:qoff + qcols],
                        ob[:].rearrange("p g e -> p (g e)"),
                    )

                    coff += cols
                    qoff += qcols

    nc.compile()
    return nc


# ---------------------------------------------------------------------------
# Entry point
# ---------------------------------------------------------------------------

_last_results = None  # BassKernelResults of the most recent run (for profiling)


def kernel(rel, pattern, w_attn, src, dst, **_unused):
    rel = np.ascontiguousarray(np.asarray(rel, dtype=np.float32))
    pattern = np.ascontiguousarray(np.asarray(pattern, dtype=np.float32))
    w_attn = np.ascontiguousarray(np.asarray(w_attn, dtype=np.float32))
    src = np.asarray(src).astype(np.int64)
    dst = np.asarray(dst).astype(np.int64)

    prep = _host_prep(rel, pattern, w_attn, src, dst, NCORES)
    sgs = prep["sgs"]

    nc = _build_program(sgs)

    in_maps = []
    for c in range(NCORES):
        pc = prep["cores"][c]
        in_maps.append(
            dict(
                hsrcw=pc["hsrcw"],
                patt=pc["patt"],
                relq=pc["relq"],
                relout=pc["relout"],
                wattn=w_attn,
            )
        )

    res = run_bass_kernel_spmd(nc, in_maps, core_ids=list(range(NCORES)))
    global _last_results
    _last_results = res

    out = np.empty((rel.shape[0], D), dtype=np.float32)
    for c in range(NCORES):
        pc = prep["cores"][c]
        oarr = res.results[c]["out"]
        qoff = 0
        for si, (_, G, K) in enumerate(sgs):
            ov = oarr[:, qoff:qoff + G * D].reshape(P, G, D).astype(np.float32)
            nd = pc["nodes"][si]
            valid = nd >= 0
            out[nd[valid]] = ov[valid]
            qoff += G * D
    return out


# revision 5
# speedup vs baseline: 2.1852x; 1.2129x over previous
"""GNN edge-softmax message-passing kernel for 8 Trainium2 NeuronCores.

Problem (see reference):
    z1 = rel[src] * pattern                       # [E, D]
    e  = leaky_relu(z1 @ w1 + rel[dst] @ w2)      # [E]
    alpha = segment_softmax(e, by dst)            # [E]
    agg   = segment_sum(alpha[:, None] * z1, dst) # [N, D]
    out   = where(deg > 0, agg, rel)

Sharding strategy (dst-ownership, no collectives):
    Every dst node is assigned to one (core, block, partition) slot.
    Nodes are degree-sorted and packed into 128-node blocks so all nodes
    in a block share the same padded edge count K; blocks are dealt
    round-robin to the 8 cores so all cores run one compiled program.
    Blocks of equal-ish K are fused into supergroups of G blocks
    (G*K <= GKMAX) so device instructions are few and large.

Device data layout ("layout B", k innermost):
    slab[p, g, d, k] fp16 for both gathered-node and pattern slabs, so
    every bulk DVE op keeps a packed (stride-1) innermost dim and runs
    in the 2x half-precision mode. All reductions are computed as
    tensor_tensor halving trees (2x mode) instead of tensor_reduce
    (1x mode only, slower still on strided views).

Algebra: w1 is folded into the node table before the host gather
    (hsrcw = (rel*w1)[src]), so  zw = hsrcw*patt  yields logits by a
    d-tree, and the weighted message sum reuses zw:
        agg' = sum_k alpha * zw = w1 * agg,
    un-scaled at the end by 1/w1 on the [P,G,64] result. Relative fp16
    error is invariant to the w1 scaling.

Pad slots are poisoned on host (hsrcw[d0] = -3000, patt[d0] = 1) so pad
    logits are ~-3000, leaky-relu -> -30, exp -> fp16 exact 0: no masks,
    no segment max (logits are O(5), exp cannot overflow), no degree
    correction. Zero-in-degree rows come out all-zero and the DGL
    fallback is a single add of a host-prepared `relout` (= rel where
    deg==0 else 0).

The scalar (ACT) engine runs leaky-relu(+q bias) and exp.
"""

import math
import numpy as np

import concourse.bacc as bacc
import concourse.tile as tile
from concourse import mybir
from concourse.bass_utils import run_bass_kernel_spmd

P = 128
NCORES = 8
D = 64
GKMAX = 192

f32 = mybir.dt.float32
f16 = mybir.dt.float16


# ---------------------------------------------------------------------------
# Host-side preprocessing
# ---------------------------------------------------------------------------

def _host_prep(rel, pattern, w_attn, src, dst, ncores):
    N = rel.shape[0]
    E = src.shape[0]

    deg = np.bincount(dst, minlength=N).astype(np.int64)
    node_order = np.argsort(-deg, kind="stable")

    group = P * ncores
    B = int(math.ceil(N / group))
    total_slots = B * group

    slot_node = np.full(total_slots, -1, dtype=np.int64)
    slot_node[:N] = node_order
    deg_slot = np.zeros(total_slots, dtype=np.int64)
    deg_slot[:N] = deg[node_order]
    Ks = deg_slot.reshape(B, group).max(axis=1).astype(np.int64)

    # supergroups of consecutive blocks, padded to the first (max) K
    sgs = []  # (jstart, G, K)
    j = 0
    while j < B:
        # K rounded up to a multiple of 4 keeps every tree-fold slice
        # 4-byte aligned (fp16), which the DVE 2x mode requires.
        K = max(4 * ((int(Ks[j]) + 3) // 4), 4)
        G = 1
        while j + G < B and (G + 1) * K <= GKMAX:
            G += 1
        sgs.append((j, G, K))
        j += G

    # per-edge coordinates (edges sorted by dst slot, k within node)
    slot_of_node = np.empty(N, dtype=np.int64)
    slot_of_node[node_order] = np.arange(N)
    e_slot = slot_of_node[dst]
    order = np.argsort(e_slot, kind="stable")
    es = e_slot[order]
    counts = np.bincount(e_slot, minlength=total_slots)
    starts = np.concatenate([[0], np.cumsum(counts)[:-1]])
    k_all = np.arange(E, dtype=np.int64) - starts[es]
    gg = es // P
    p_all = es % P
    c_all = (gg % ncores).astype(np.int64)
    j_all = gg // ncores
    src_all = src[order]
    prow_all = order

    relw = (rel * w_attn[None, :D]).astype(np.float32)  # w1 folded into table

    cores = []
    for c in range(ncores):
        mc = c_all == c
        hs_parts, pt_parts, rq_parts, ro_parts = [], [], [], []
        nodes_parts = []
        for (j0, G, K) in sgs:
            msk = mc & (j_all >= j0) & (j_all < j0 + G)
            pe = p_all[msk]
            ge = j_all[msk] - j0
            ke = k_all[msk]

            hv = np.zeros((P, G, D, K), dtype=np.float16)
            pv = np.zeros((P, G, D, K), dtype=np.float16)
            hv[pe, ge, :, ke] = relw[src_all[msk]]
            pv[pe, ge, :, ke] = pattern[prow_all[msk]]

            slots = ((j0 + np.arange(G)[None, :]) * ncores + c) * P \
                + np.arange(P)[:, None]                      # [P, G]
            nd = slot_node[slots]
            dg = deg_slot[slots]
            pmask = np.arange(K)[None, None, :] >= dg[:, :, None]
            pi, gi, ki = np.nonzero(pmask)
            hv[pi, gi, 0, ki] = -3000.0
            pv[pi, gi, 0, ki] = 1.0

            qv = np.zeros((P, G, D), dtype=np.float16)
            ov = np.zeros((P, G, D), dtype=np.float16)
            valid = nd >= 0
            qv[valid] = rel[nd[valid]]
            zd = valid & (dg == 0)
            ov[zd] = rel[nd[zd]]

            hs_parts.append(hv.reshape(P, -1))
            pt_parts.append(pv.reshape(P, -1))
            rq_parts.append(qv.reshape(P, -1))
            ro_parts.append(ov.reshape(P, -1))
            nodes_parts.append(nd)

        cores.append(
            dict(
                hsrcw=np.ascontiguousarray(np.concatenate(hs_parts, axis=1)),
                patt=np.ascontiguousarray(np.concatenate(pt_parts, axis=1)),
                relq=np.ascontiguousarray(np.concatenate(rq_parts, axis=1)),
                relout=np.ascontiguousarray(np.concatenate(ro_parts, axis=1)),
                nodes=nodes_parts,
            )
        )

    return dict(cores=cores, sgs=sgs)


# ---------------------------------------------------------------------------
# Device program
# ---------------------------------------------------------------------------

def _build_program(sgs, d=D):
    total_cols = sum(G * d * K for (_, G, K) in sgs)
    totq = sum(G * d for (_, G, _) in sgs)

    nc = bacc.Bacc("TRN2", target_bir_lowering=False)

    hsrcw_t = nc.dram_tensor("hsrcw", [P, total_cols], f16, kind="ExternalInput")
    patt_t = nc.dram_tensor("patt", [P, total_cols], f16, kind="ExternalInput")
    relq_t = nc.dram_tensor("relq", [P, totq], f16, kind="ExternalInput")
    relout_t = nc.dram_tensor("relout", [P, totq], f16, kind="ExternalInput")
    wattn_t = nc.dram_tensor("wattn", [2 * d], f32, kind="ExternalInput")
    out_t = nc.dram_tensor("out", [P, totq], f16, kind="ExternalOutput")

    mult = mybir.AluOpType.mult
    add = mybir.AluOpType.add
    mx = mybir.AluOpType.max
    X = mybir.AxisListType.X
    Lrelu = mybir.ActivationFunctionType.Lrelu
    Exp = mybir.ActivationFunctionType.Exp

    with tile.TileContext(nc) as tc:
        with (
            tc.tile_pool(name="const", bufs=1) as cpool,
            tc.tile_pool(name="big", bufs=2) as bpool,
            tc.tile_pool(name="small", bufs=2) as spool,
        ):
            w_row = cpool.tile([1, 2 * d], f32, tag="w_row")
            nc.sync.dma_start(w_row[:], wattn_t[:].rearrange("(p f) -> p f", p=1))
            w_all = cpool.tile([P, 2 * d], f32, tag="w_all")
            nc.gpsimd.partition_broadcast(w_all[:], w_row[:])
            w2_16 = cpool.tile([P, 1, d], f16, tag="w2_16")
            nc.vector.tensor_copy(w2_16[:], w_all[:, d:].unsqueeze(1))
            w1inv = cpool.tile([P, 1, d], f32, tag="w1inv")
            nc.vector.reciprocal(w1inv[:], w_all[:, :d].unsqueeze(1))

            coff = 0
            qoff = 0
            with nc.allow_low_precision(reason="fp16 streams within tolerance"):
                for (j0, G, K) in sgs:
                    cols = G * d * K
                    qcols = G * d

                    hs = bpool.tile([P, G, d, K], f16, tag="hs")
                    hsf = hs[:].rearrange("p g e k -> p (g e k)")
                    for a, b in ((0, cols // 2), (cols // 2, cols)):
                        nc.sync.dma_start(hsf[:, a:b], hsrcw_t[:, coff + a:coff + b])
                    pt = bpool.tile([P, G, d, K], f16, tag="pt")
                    ptf = pt[:].rearrange("p g e k -> p (g e k)")
                    for a, b in ((0, cols // 2), (cols // 2, cols)):
                        nc.sync.dma_start(ptf[:, a:b], patt_t[:, coff + a:coff + b])
                    rq = spool.tile([P, G, d], f16, tag="rq")
                    nc.sync.dma_start(
                        rq[:].rearrange("p g e -> p (g e)"),
                        relq_t[:, qoff:qoff + qcols],
                    )
                    ro = spool.tile([P, G, d], f16, tag="ro")
                    nc.sync.dma_start(
                        ro[:].rearrange("p g e -> p (g e)"),
                        relout_t[:, qoff:qoff + qcols],
                    )

                    # q = (relq * w2) summed over d: multiply on gpsimd,
                    # per-block accumulate on the scalar engine — both idle,
                    # and early so the bias is ready before the lrelu.
                    qt = spool.tile([P, G, d], f16, tag="qt")
                    nc.gpsimd.tensor_tensor(
                        out=qt[:], in0=rq[:], in1=w2_16[:].to_broadcast([P, G, d]),
                        op=mult,
                    )
                    qq = spool.tile([P, G], f32, tag="qq")
                    for g in range(G):
                        nc.scalar.activation(
                            out=qt[:, g, :], in_=qt[:, g, :],
                            func=mybir.ActivationFunctionType.Copy,
                            accum_out=qq[:, g:g + 1],
                        )

                    # zw = hsrcw * patt, in place over hs
                    nc.vector.tensor_tensor(out=hs[:], in0=hs[:], in1=pt[:], op=mult)

                    # logits = sum_d zw : halving tree over the d axis.
                    # level 1 -> scratch (zw must survive); further in place.
                    lt = bpool.tile([P, G, d // 2, K], f16, tag="lt")
                    nc.vector.tensor_tensor(
                        out=lt[:], in0=hs[:, :, :d // 2, :], in1=hs[:, :, d // 2:, :],
                        op=add,
                    )
                    w = d // 2
                    while w > 1:
                        h = w // 2
                        nc.vector.tensor_tensor(
                            out=lt[:, :, :h, :], in0=lt[:, :, :h, :],
                            in1=lt[:, :, h:2 * h, :], op=add,
                        )
                        w = h

                    # ex = exp(leaky_relu(logits + q)); pads underflow to 0
                    el = spool.tile([P, G, K], f16, tag="el")
                    for g in range(G):
                        nc.scalar.activation(
                            out=el[:, g, :], in_=lt[:, g, 0, :], func=Lrelu,
                            bias=qq[:, g:g + 1], alpha=0.01,
                        )
                    ex = spool.tile([P, G, K], f16, tag="ex")
                    nc.scalar.activation(
                        out=ex[:].rearrange("p g k -> p (g k)"),
                        in_=el[:].rearrange("p g k -> p (g k)"), func=Exp,
                    )

                    # alpha = ex / sum_k ex   (in place over ex)
                    sc = spool.tile([P, G], f32, tag="sc")
                    nc.vector.tensor_reduce(out=sc[:], in_=ex[:], axis=X, op=add)
                    scl = spool.tile([P, G], f32, tag="scl")
                    nc.vector.tensor_scalar(
                        out=scl[:], in0=sc[:], scalar1=1e-30, scalar2=None, op0=mx
                    )
                    rc = spool.tile([P, G], f32, tag="rc")
                    nc.vector.reciprocal(rc[:], scl[:])
                    nc.vector.tensor_tensor(
                        out=ex[:], in0=ex[:],
                        in1=rc[:].unsqueeze(2).to_broadcast([P, G, K]), op=mult,
                    )

                    # ext = zw * alpha, in place over pt; then k tree that
                    # folds the tail onto the largest power of two below w,
                    # so every fold slice stays 4-byte aligned (2x mode).
                    nc.vector.tensor_tensor(
                        out=pt[:], in0=hs[:],
                        in1=ex[:].unsqueeze(2).to_broadcast([P, G, d, K]), op=mult,
                    )
                    w = K
                    while w > 1:
                        a = w // 2 if (w & (w - 1)) == 0 else 1 << (w.bit_length() - 1)
                        nc.vector.tensor_tensor(
                            out=pt[:, :, :, :w - a], in0=pt[:, :, :, :w - a],
                            in1=pt[:, :, :, a:w], op=add,
                        )
                        w = a

                    # agg = agg' / w1 ; out = agg + relout  (both on gpsimd)
                    ag = spool.tile([P, G, d], f32, tag="ag")
                    nc.gpsimd.tensor_tensor(
                        out=ag[:], in0=pt[:, :, :, 0],
                        in1=w1inv[:].to_broadcast([P, G, d]), op=mult,
                    )
                    ob = spool.tile([P, G, d], f16, tag="ob")
                    nc.gpsimd.tensor_tensor(out=ob[:], in0=ag[:], in1=ro[:], op=add)
                    nc.sync.dma_start(
                        out_t[:, qoff:qoff + qcols],
                        ob[:].rearrange("p g e -> p (g e)"),
                    )

                    coff += cols
                    qoff += qcols

    nc.compile()
    return nc


# ---------------------------------------------------------------------------
# Entry point
# ---------------------------------------------------------------------------

_last_results = None  # BassKernelResults of the most recent run (for profiling)


def kernel(rel, pattern, w_attn, src, dst, **_unused):
    rel = np.ascontiguousarray(np.asarray(rel, dtype=np.float32))
    pattern = np.ascontiguousarray(np.asarray(pattern, dtype=np.float32))
    w_attn = np.ascontiguousarray(np.asarray(w_attn, dtype=np.float32))
    src = np.asarray(src).astype(np.int64)
    dst = np.asarray(dst).astype(np.int64)

    prep = _host_prep(rel, pattern, w_attn, src, dst, NCORES)
    sgs = prep["sgs"]

    nc = _build_program(sgs)

    in_maps = []
    for c in range(NCORES):
        pc = prep["cores"][c]
        in_maps.append(
            dict(
                hsrcw=pc["hsrcw"],
                patt=pc["patt"],
                relq=pc["relq"],
                relout=pc["relout"],
                wattn=w_attn,
            )
        )

    res = run_bass_kernel_spmd(nc, in_maps, core_ids=list(range(NCORES)))
    global _last_results
    _last_results = res

    out = np.empty((rel.shape[0], D), dtype=np.float32)
    for c in range(NCORES):
        pc = prep["cores"][c]
        oarr = res.results[c]["out"]
        qoff = 0
        for si, (_, G, K) in enumerate(sgs):
            ov = oarr[:, qoff:qoff + G * D].reshape(P, G, D).astype(np.float32)
            nd = pc["nodes"][si]
            valid = nd >= 0
            out[nd[valid]] = ov[valid]
            qoff += G * D
    return out


# revision 7
# speedup vs baseline: 2.3786x; 1.0885x over previous
"""GNN edge-softmax message-passing kernel for 8 Trainium2 NeuronCores.

Problem (see reference):
    z1 = rel[src] * pattern                       # [E, D]
    e  = leaky_relu(z1 @ w1 + rel[dst] @ w2)      # [E]
    alpha = segment_softmax(e, by dst)            # [E]
    agg   = segment_sum(alpha[:, None] * z1, dst) # [N, D]
    out   = where(deg > 0, agg, rel)

Sharding strategy (dst-ownership, no collectives):
    Every dst node is assigned to one (core, block, partition) slot.
    Nodes are degree-sorted and packed into 128-node blocks so all nodes
    in a block share the same padded edge count K; blocks are dealt
    round-robin to the 8 cores so all cores run one compiled program.
    Blocks of equal-ish K are fused into supergroups of G blocks
    (G*K <= GKMAX) so device instructions are few and large.

Device data layout ("layout B", k innermost):
    slab[p, g, d, k] fp16 for both gathered-node and pattern slabs, so
    every bulk DVE op keeps a packed (stride-1) innermost dim and runs
    in the 2x half-precision mode. All reductions are computed as
    tensor_tensor halving trees (2x mode) instead of tensor_reduce
    (1x mode only, slower still on strided views).

Algebra: w1 is folded into the node table before the host gather
    (hsrcw = (rel*w1)[src]), so  zw = hsrcw*patt  yields logits by a
    d-tree, and the weighted message sum reuses zw:
        agg' = sum_k alpha * zw = w1 * agg,
    un-scaled at the end by 1/w1 on the [P,G,64] result. Relative fp16
    error is invariant to the w1 scaling.

Pad slots are poisoned on host (hsrcw[d0] = -3000, patt[d0] = 1) so pad
    logits are ~-3000, leaky-relu -> -30, exp -> fp16 exact 0: no masks,
    no segment max (logits are O(5), exp cannot overflow), no degree
    correction. Zero-in-degree rows come out all-zero and the DGL
    fallback is a single add of a host-prepared `relout` (= rel where
    deg==0 else 0).

The scalar (ACT) engine runs leaky-relu(+q bias) and exp.
"""

import math
import numpy as np

import concourse.bacc as bacc
import concourse.tile as tile
from concourse import mybir
from concourse.bass_utils import run_bass_kernel_spmd

P = 128
NCORES = 8
D = 64
GKMAX = 160

f32 = mybir.dt.float32
f16 = mybir.dt.float16


# ---------------------------------------------------------------------------
# Host-side preprocessing
# ---------------------------------------------------------------------------

def _host_prep(rel, pattern, w_attn, src, dst, ncores):
    N = rel.shape[0]
    E = src.shape[0]

    deg = np.bincount(dst, minlength=N).astype(np.int64)
    node_order = np.argsort(-deg, kind="stable")

    group = P * ncores
    B = int(math.ceil(N / group))
    total_slots = B * group

    slot_node = np.full(total_slots, -1, dtype=np.int64)
    slot_node[:N] = node_order
    deg_slot = np.zeros(total_slots, dtype=np.int64)
    deg_slot[:N] = deg[node_order]
    Ks = deg_slot.reshape(B, group).max(axis=1).astype(np.int64)

    # supergroups of consecutive blocks, padded to the first (max) K
    sgs = []  # (jstart, G, K)
    j = 0
    while j < B:
        # K rounded up to a multiple of 4 keeps every tree-fold slice
        # 4-byte aligned (fp16), which the DVE 2x mode requires.
        K = max(4 * ((int(Ks[j]) + 3) // 4), 4)
        G = 1
        while j + G < B and (G + 1) * K <= GKMAX:
            G += 1
        sgs.append((j, G, K))
        j += G

    # per-edge coordinates (edges sorted by dst slot, k within node)
    slot_of_node = np.empty(N, dtype=np.int64)
    slot_of_node[node_order] = np.arange(N)
    e_slot = slot_of_node[dst]
    order = np.argsort(e_slot, kind="stable")
    es = e_slot[order]
    counts = np.bincount(e_slot, minlength=total_slots)
    starts = np.concatenate([[0], np.cumsum(counts)[:-1]])
    k_all = np.arange(E, dtype=np.int64) - starts[es]
    gg = es // P
    p_all = es % P
    c_all = (gg % ncores).astype(np.int64)
    j_all = gg // ncores
    src_all = src[order]
    prow_all = order

    relw = (rel * w_attn[None, :D]).astype(np.float32)   # w1 folded into table
    relw2 = (rel * w_attn[None, D:]).astype(np.float32)  # w2 folded into table

    cores = []
    for c in range(ncores):
        mc = c_all == c
        hs_parts, pt_parts, rq_parts, ro_parts = [], [], [], []
        nodes_parts = []
        for (j0, G, K) in sgs:
            msk = mc & (j_all >= j0) & (j_all < j0 + G)
            pe = p_all[msk]
            ge = j_all[msk] - j0
            ke = k_all[msk]

            hv = np.zeros((P, G, D, K), dtype=np.float16)
            pv = np.zeros((P, G, D, K), dtype=np.float16)
            hv[pe, ge, :, ke] = relw[src_all[msk]]
            pv[pe, ge, :, ke] = pattern[prow_all[msk]]

            slots = ((j0 + np.arange(G)[None, :]) * ncores + c) * P \
                + np.arange(P)[:, None]                      # [P, G]
            nd = slot_node[slots]
            dg = deg_slot[slots]
            pmask = np.arange(K)[None, None, :] >= dg[:, :, None]
            pi, gi, ki = np.nonzero(pmask)
            hv[pi, gi, 0, ki] = -3000.0
            pv[pi, gi, 0, ki] = 1.0

            qv = np.zeros((P, G, D), dtype=np.float16)
            ov = np.zeros((P, G, D), dtype=np.float16)
            valid = nd >= 0
            qv[valid] = relw2[nd[valid]]
            zd = valid & (dg == 0)
            ov[zd] = rel[nd[zd]]

            hs_parts.append(hv.reshape(P, -1))
            pt_parts.append(pv.reshape(P, -1))
            rq_parts.append(qv.reshape(P, -1))
            ro_parts.append(ov.reshape(P, -1))
            nodes_parts.append(nd)

        cores.append(
            dict(
                hsrcw=np.ascontiguousarray(np.concatenate(hs_parts, axis=1)),
                patt=np.ascontiguousarray(np.concatenate(pt_parts, axis=1)),
                relq=np.ascontiguousarray(np.concatenate(rq_parts, axis=1)),
                relout=np.ascontiguousarray(np.concatenate(ro_parts, axis=1)),
                nodes=nodes_parts,
            )
        )

    return dict(cores=cores, sgs=sgs)


# ---------------------------------------------------------------------------
# Device program
# ---------------------------------------------------------------------------

def _build_program(sgs, d=D):
    total_cols = sum(G * d * K for (_, G, K) in sgs)
    totq = sum(G * d for (_, G, _) in sgs)

    nc = bacc.Bacc("TRN2", target_bir_lowering=False)

    hsrcw_t = nc.dram_tensor("hsrcw", [P, total_cols], f16, kind="ExternalInput")
    patt_t = nc.dram_tensor("patt", [P, total_cols], f16, kind="ExternalInput")
    relq_t = nc.dram_tensor("relq", [P, totq], f16, kind="ExternalInput")
    relout_t = nc.dram_tensor("relout", [P, totq], f16, kind="ExternalInput")
    wattn_t = nc.dram_tensor("wattn", [2 * d], f32, kind="ExternalInput")
    out_t = nc.dram_tensor("out", [P, totq], f16, kind="ExternalOutput")

    mult = mybir.AluOpType.mult
    add = mybir.AluOpType.add
    mx = mybir.AluOpType.max
    X = mybir.AxisListType.X
    Lrelu = mybir.ActivationFunctionType.Lrelu
    Exp = mybir.ActivationFunctionType.Exp

    with tile.TileContext(nc) as tc:
        with (
            tc.tile_pool(name="const", bufs=1) as cpool,
            tc.tile_pool(name="big", bufs=3) as bpool,
            tc.tile_pool(name="small", bufs=3) as spool,
        ):
            w_row = cpool.tile([1, 2 * d], f32, tag="w_row")
            nc.sync.dma_start(w_row[:], wattn_t[:].rearrange("(p f) -> p f", p=1))
            w_all = cpool.tile([P, 2 * d], f32, tag="w_all")
            nc.gpsimd.partition_broadcast(w_all[:], w_row[:])
            w1inv = cpool.tile([P, 1, d], f32, tag="w1inv")
            nc.vector.reciprocal(w1inv[:], w_all[:, :d].unsqueeze(1))

            coffs = np.concatenate(
                [[0], np.cumsum([G * d * K for (_, G, K) in sgs])]
            ).astype(int)
            qoffs = np.concatenate(
                [[0], np.cumsum([G * d for (_, G, _) in sgs])]
            ).astype(int)

            def emit_a(si):
                """DMAs, q accumulation (ACT), zw and the d-tree (DVE)."""
                _, G, K = sgs[si]
                cols = G * d * K
                coff = int(coffs[si])
                qoff = int(qoffs[si])

                hs = bpool.tile([P, G, d, K], f16, tag="hs")
                hsf = hs[:].rearrange("p g e k -> p (g e k)")
                for a, b in ((0, cols // 2), (cols // 2, cols)):
                    nc.sync.dma_start(hsf[:, a:b], hsrcw_t[:, coff + a:coff + b])
                pt = bpool.tile([P, G, d, K], f16, tag="pt")
                ptf = pt[:].rearrange("p g e k -> p (g e k)")
                for a, b in ((0, cols // 2), (cols // 2, cols)):
                    nc.sync.dma_start(ptf[:, a:b], patt_t[:, coff + a:coff + b])
                rq = spool.tile([P, G, d], f16, tag="rq")
                nc.sync.dma_start(
                    rq[:].rearrange("p g e -> p (g e)"),
                    relq_t[:, qoff:qoff + G * d],
                )
                ro = spool.tile([P, G, d], f16, tag="ro")
                nc.sync.dma_start(
                    ro[:].rearrange("p g e -> p (g e)"),
                    relout_t[:, qoff:qoff + G * d],
                )

                # q = sum_d relq (w2 pre-folded on host): ACT accumulators
                qq = spool.tile([P, G], f32, tag="qq")
                for g in range(G):
                    nc.scalar.activation(
                        out=rq[:, g, :], in_=rq[:, g, :],
                        func=mybir.ActivationFunctionType.Copy,
                        accum_out=qq[:, g:g + 1],
                    )

                # zw = hsrcw * patt, in place over hs
                nc.vector.tensor_tensor(out=hs[:], in0=hs[:], in1=pt[:], op=mult)

                # logits = sum_d zw : halving tree over the d axis.
                # level 1 -> scratch (zw must survive); further in place.
                lt = bpool.tile([P, G, d // 2, K], f16, tag="lt")
                nc.vector.tensor_tensor(
                    out=lt[:], in0=hs[:, :, :d // 2, :], in1=hs[:, :, d // 2:, :],
                    op=add,
                )
                w = d // 2
                while w > 1:
                    h = w // 2
                    nc.vector.tensor_tensor(
                        out=lt[:, :, :h, :], in0=lt[:, :, :h, :],
                        in1=lt[:, :, h:2 * h, :], op=add,
                    )
                    w = h
                return dict(si=si, G=G, K=K, hs=hs, pt=pt, lt=lt, ro=ro, qq=qq)

            def emit_b(st):
                """Softmax, weighted aggregation and output of a group."""
                si, G, K = st["si"], st["G"], st["K"]
                hs, pt, lt, ro, qq = st["hs"], st["pt"], st["lt"], st["ro"], st["qq"]
                qoff = int(qoffs[si])

                # ex = exp(leaky_relu(logits + q)); pads underflow to 0
                el = spool.tile([P, G, K], f16, tag="el")
                for g in range(G):
                    nc.scalar.activation(
                        out=el[:, g, :], in_=lt[:, g, 0, :], func=Lrelu,
                        bias=qq[:, g:g + 1], alpha=0.01,
                    )
                ex = spool.tile([P, G, K], f16, tag="ex")
                nc.scalar.activation(
                    out=ex[:].rearrange("p g k -> p (g k)"),
                    in_=el[:].rearrange("p g k -> p (g k)"), func=Exp,
                )

                # alpha = ex / sum_k ex   (in place over ex)
                sc = spool.tile([P, G], f32, tag="sc")
                nc.vector.tensor_reduce(out=sc[:], in_=ex[:], axis=X, op=add)
                scl = spool.tile([P, G], f32, tag="scl")
                nc.vector.tensor_scalar(
                    out=scl[:], in0=sc[:], scalar1=1e-30, scalar2=None, op0=mx
                )
                rc = spool.tile([P, G], f32, tag="rc")
                nc.vector.reciprocal(rc[:], scl[:])
                nc.vector.tensor_tensor(
                    out=ex[:], in0=ex[:],
                    in1=rc[:].unsqueeze(2).to_broadcast([P, G, K]), op=mult,
                )

                # ext = zw * alpha, in place over pt; then k tree that
                # folds the tail onto the largest power of two below w,
                # so every fold slice stays 4-byte aligned (2x mode).
                nc.vector.tensor_tensor(
                    out=pt[:], in0=hs[:],
                    in1=ex[:].unsqueeze(2).to_broadcast([P, G, d, K]), op=mult,
                )
                w = K
                while w > 1:
                    a = w // 2 if (w & (w - 1)) == 0 else 1 << (w.bit_length() - 1)
                    nc.vector.tensor_tensor(
                        out=pt[:, :, :, :w - a], in0=pt[:, :, :, :w - a],
                        in1=pt[:, :, :, a:w], op=add,
                    )
                    w = a

                # agg = agg' / w1 ; out = agg + relout  (both on gpsimd)
                ag = spool.tile([P, G, d], f32, tag="ag")
                nc.gpsimd.tensor_tensor(
                    out=ag[:], in0=pt[:, :, :, 0],
                    in1=w1inv[:].to_broadcast([P, G, d]), op=mult,
                )
                ob = spool.tile([P, G, d], f16, tag="ob")
                nc.gpsimd.tensor_tensor(out=ob[:], in0=ag[:], in1=ro[:], op=add)
                nc.sync.dma_start(
                    out_t[:, qoff:qoff + G * d],
                    ob[:].rearrange("p g e -> p (g e)"),
                )

            # software pipeline: group i+1's pre-ACT stage is emitted before
            # group i's post-ACT stage so the DVE never waits on the scalar
            # engine's lrelu/exp round trip.
            with nc.allow_low_precision(reason="fp16 streams within tolerance"):
                prev = emit_a(0)
                for si in range(1, len(sgs)):
                    cur = emit_a(si)
                    emit_b(prev)
                    prev = cur
                emit_b(prev)

    nc.compile()
    return nc


# ---------------------------------------------------------------------------
# Entry point
# ---------------------------------------------------------------------------

_last_results = None  # BassKernelResults of the most recent run (for profiling)


def kernel(rel, pattern, w_attn, src, dst, **_unused):
    rel = np.ascontiguousarray(np.asarray(rel, dtype=np.float32))
    pattern = np.ascontiguousarray(np.asarray(pattern, dtype=np.float32))
    w_attn = np.ascontiguousarray(np.asarray(w_attn, dtype=np.float32))
    src = np.asarray(src).astype(np.int64)
    dst = np.asarray(dst).astype(np.int64)

    prep = _host_prep(rel, pattern, w_attn, src, dst, NCORES)
    sgs = prep["sgs"]

    nc = _build_program(sgs)

    in_maps = []
    for c in range(NCORES):
        pc = prep["cores"][c]
        in_maps.append(
            dict(
                hsrcw=pc["hsrcw"],
                patt=pc["patt"],
                relq=pc["relq"],
                relout=pc["relout"],
                wattn=w_attn,
            )
        )

    res = run_bass_kernel_spmd(nc, in_maps, core_ids=list(range(NCORES)))
    global _last_results
    _last_results = res

    out = np.empty((rel.shape[0], D), dtype=np.float32)
    for c in range(NCORES):
        pc = prep["cores"][c]
        oarr = res.results[c]["out"]
        qoff = 0
        for si, (_, G, K) in enumerate(sgs):
            ov = oarr[:, qoff:qoff + G * D].reshape(P, G, D).astype(np.float32)
            nd = pc["nodes"][si]
            valid = nd >= 0
            out[nd[valid]] = ov[valid]
            qoff += G * D
    return out
